# revision 55
# baseline (speedup 1.0000x reference)
"""Trainium2 Bass kernel for MultiHeadAttention with relative position bias.

Reference computation (B=2, S=2048, D=1024, H=16, Dk=64, MAX_REL=128):
    Q,K,V = x@W{q,k,v}.T + b      (per-head reshape)
    scores = QK^T/sqrt(Dk) + rel_bias_matrix
    out = softmax(scores) @ V, heads merged, @ Wo.T + bo

Sharding (8 cores): core c handles batch b=c//4 and 4 heads hg=4*(c%4)..+4
(data + head parallel). Q/K/V projections column-split per head group,
Wo row-split; the partial outputs are summed on the host (the "all-reduce").

Pipeline design:
  The exp stream on the Scalar/ACT engine (16.8M elem/core at 1 elem/
  cycle/lane @1.2GHz, ~142us) and the PE matmul stream (~200us incl the
  per-MM weight-load tax) are the two co-bottlenecks; the schedule keeps
  both fed.  All tensor data is bf16 (final rel err ~6e-3 vs 2e-2
  tolerance): halves input DMA, enables FWL weight loads, shrinks SBUF.
  PSUM budget: stp pool (2x[128,1024] slots, 4 banks) for QK scores +
  Wo tiles; accp pool (2 slots, 4 banks) for projection flights + PV
  accumulators.

  Emission order: input DMAs (xt split into 16 chunk DMAs - one
  dma_start lands on ~one queue at ~24GB/s, so parallelism needs many
  in flight) -> K0/Q0 flights -> group0 QK+exp (pt tiles buffered in a
  17-deep pool) -> K1/Q1/V flights -> group0 PV + normalize -> groups
  1-3 fully interleaved -> transposed-Wo phase.  Group order qh-major.

  Softmax trick: P~ = exp(s/8) is the softmax numerator up to the
  per-head constant e^{-c_past} which cancels in the normalize; the
  "future" region (q-k <= -128) gets a constant multiplier on DVE and
  the 384-wide Toeplitz band a host-precomputed exp(bias - c_past)
  tile.  V carries a ones column so PV yields the denominator for free;
  normalize uses reciprocal_approx_fast (5x DVE reciprocal) on a
  DRAM-broadcast denominator.  Output is emitted transposed [D,S] in
  bf16 (Wo stationary => LDW amortized; halved out-DMA) and the host
  upcasts/transposes/sums.
"""

import math
import os
import sys

for _p in ("/opt/trn_rl_repo", "/root/.axon_site", "/root/.axon_site/_ro/trn_rl_repo",
           "/root/.axon_site/_ro/pypackages"):
    if os.path.isdir(_p) and _p not in sys.path:
        sys.path.append(_p)

import numpy as np
import ml_dtypes

import concourse.bass as bass
import concourse.mybir as mybir
import concourse.tile as tile
from concourse import bacc
from contextlib import ExitStack

# Problem constants (hardcoded per the contract).
B, S, D = 2, 2048, 1024
H, DK = 16, 64
MAX_REL = 128
N_CORES = 8
CORES_PER_BATCH = 4
HEADS_PER_CORE = H // CORES_PER_BATCH  # 4
CL = HEADS_PER_CORE * DK               # 256 local channels
N_PAIRS = HEADS_PER_CORE // 2          # 2 head pairs
QH = 1024                              # q processed in halves
N_QH = S // QH                         # 2
N_KC = S // 128                        # 16 k chunks
NDC = D // 128                         # 8 contraction chunks
BAND = 3 * 128                         # band width in q for one k chunk

F32 = mybir.dt.float32
BF16 = mybir.dt.bfloat16

SCALE = 1.0 / math.sqrt(DK)
EXP = mybir.ActivationFunctionType.Exp


def build_program():
    nc = bacc.Bacc("TRN2", target_bir_lowering=False, debug=False)

    xt_d = nc.declare_dram_parameter("xt", [D, S], BF16, isOutput=False)
    wqt_d = nc.declare_dram_parameter("wqt", [D, CL], BF16, isOutput=False)
    wkt_d = nc.declare_dram_parameter("wkt", [D, CL], BF16, isOutput=False)
    wvt_d = nc.declare_dram_parameter("wvt", [D, CL], BF16, isOutput=False)
    wot_d = nc.declare_dram_parameter("wot", [CL, D], BF16, isOutput=False)
    bqk_d = nc.declare_dram_parameter("bqk", [128, 4], F32, isOutput=False)
    band_d = nc.declare_dram_parameter("band", [128, HEADS_PER_CORE, BAND], BF16,
                                       isOutput=False)
    # future-region multiplier exp(c_fut - c_past), replicated over partitions
    fmult_d = nc.declare_dram_parameter("fmult", [128, HEADS_PER_CORE], F32,
                                        isOutput=False)
    # partial output, TRANSPOSED [D, S] in bf16 (host upcasts + transposes)
    out_d = nc.declare_dram_parameter("out_p", [D, S], BF16, isOutput=True)
    # denominator round-trip scratch: [group, hh, q]
    den_d = nc.dram_tensor("den_scratch", [2 * N_PAIRS * N_QH, 2, QH], F32)

    with tile.TileContext(nc) as tc, ExitStack() as ctx:
        # ---------- long-lived SBUF ----------
        persist = ctx.enter_context(tc.tile_pool(name="persist", bufs=1))
        q_sb = persist.tile([128, 2, S], BF16, tag="q_sb")
        k_sb = persist.tile([128, 2, S], BF16, tag="k_sb")
        v_sb = persist.tile([128, N_KC, HEADS_PER_CORE, DK + 1], BF16, tag="v_sb")
        ct_sb = persist.tile([128, 2, S], BF16, tag="ct_sb")
        wo_sb = persist.tile([128, 2, D], BF16, tag="wo_sb")
        band_sb = persist.tile([128, HEADS_PER_CORE, BAND], BF16, tag="band_sb")
        bqk_sb = persist.tile([128, 4], F32, tag="bqk_sb")
        fmult_sb = persist.tile([128, HEADS_PER_CORE], F32, tag="fmult_sb")
        warm_sb = persist.tile([128, 16], F32, tag="warm_sb")

        xw = ctx.enter_context(tc.tile_pool(name="xw", bufs=1))
        xt_sb = xw.tile([128, NDC, S], BF16, tag="xt_sb")
        wq_sb = xw.tile([128, NDC, CL], BF16, tag="wq_sb")
        wk_sb = xw.tile([128, NDC, CL], BF16, tag="wk_sb")
        wv_sb = xw.tile([128, NDC, CL], BF16, tag="wv_sb")

        nrm = ctx.enter_context(tc.tile_pool(name="nrm", bufs=2))
        ptp = ctx.enter_context(tc.tile_pool(name="ptp", bufs=18))
        outp = ctx.enter_context(tc.tile_pool(name="outp", bufs=3))

        # ---------- PSUM pools: 4 banks each ----------
        stp = ctx.enter_context(tc.tile_pool(name="stp", bufs=2, space="PSUM"))
        accp = ctx.enter_context(tc.tile_pool(name="accp", bufs=2, space="PSUM"))

        # ---------- input DMAs, interleaved for early start ----------
        xt_v = xt_d.ap().rearrange("(c p) s -> p c s", p=128)
        wq_v = wqt_d.ap().rearrange("(c p) m -> p c m", p=128)
        wk_v = wkt_d.ap().rearrange("(c p) m -> p c m", p=128)
        wv_v = wvt_d.ap().rearrange("(c p) m -> p c m", p=128)

        # ACT exp-table warmup (overlaps the input DMA wait)
        nc.vector.memset(warm_sb, 0.0)
        nc.scalar.activation(out=warm_sb, in_=warm_sb, func=EXP, scale=1.0)

        # weights first (small), then xt split into 16 half-chunk DMAs so all
        # 16 DMA queues stream in parallel (~24 GB/s per queue)
        nc.sync.dma_start(out=wk_sb, in_=wk_v)
        nc.sync.dma_start(out=wq_sb, in_=wq_v)
        for dc in range(NDC):
            for h in range(2):
                nc.sync.dma_start(out=xt_sb[:, dc, h * 1024:(h + 1) * 1024],
                                  in_=xt_v[:, dc, h * 1024:(h + 1) * 1024])
        nc.sync.dma_start(out=wv_sb, in_=wv_v)
        nc.sync.dma_start(out=band_sb, in_=band_d.ap())
        nc.sync.dma_start(out=bqk_sb, in_=bqk_d.ap())
        nc.sync.dma_start(out=fmult_sb, in_=fmult_d.ap())
        nc.sync.dma_start(out=wo_sb, in_=wot_d.ap().rearrange("(c p) m -> p c m", p=128))
        nc.vector.memset(v_sb[:, :, :, DK:DK + 1], 1.0)

        # ---------- projection flights (dc-outer: LDW amortized 4x) ----------
        def qk_flight_t(w_sb, dst_sb, j, bias_col, t):
            """One s-half of a Q/K projection in a single PSUM slot: K0-t0
            and Q0-t0 pace concurrently on the incoming xt chunk stream
            (startup only; K1/Q1 use the whole flights below)."""
            slot = accp.tile([128, 1024], F32, tag="acc")
            for dc in range(NDC):
                for half in range(2):
                    nc.tensor.matmul(
                        slot[:, half * 512:(half + 1) * 512],
                        lhsT=w_sb[:, dc, j * 128:(j + 1) * 128],
                        rhs=xt_sb[:, dc, t * 1024 + half * 512:
                                  t * 1024 + (half + 1) * 512],
                        start=(dc == 0), stop=(dc == NDC - 1),
                    )
            nc.vector.tensor_scalar_add(
                out=dst_sb[:, j, t * 1024:(t + 1) * 1024],
                in0=slot,
                scalar1=bqk_sb[:, bias_col:bias_col + 1],
            )

        def qk_flight(w_sb, dst_sb, j, bias_col):
            slot_a = accp.tile([128, 1024], F32, tag="acc")
            slot_b = accp.tile([128, 1024], F32, tag="acc")
            slots = [slot_a, slot_b]
            for dc in range(NDC):
                for t in range(2):
                    for half in range(2):
                        nc.tensor.matmul(
                            slots[t][:, half * 512:(half + 1) * 512],
                            lhsT=w_sb[:, dc, j * 128:(j + 1) * 128],
                            rhs=xt_sb[:, dc, t * 1024 + half * 512:
                                      t * 1024 + (half + 1) * 512],
                            start=(dc == 0), stop=(dc == NDC - 1),
                        )
            for t in range(2):
                nc.vector.tensor_scalar_add(
                    out=dst_sb[:, j, t * 1024:(t + 1) * 1024],
                    in0=slots[t],
                    scalar1=bqk_sb[:, bias_col:bias_col + 1],
                )

        def v_flight(scg):
            # each 256-wide accumulation group must own a full PSUM bank
            # (start=True clears has_written for the whole bank), so 4
            # s-chunks land at 512-col boundaries across two slots.
            slot_a = accp.tile([128, 1024], F32, tag="acc")
            slot_b = accp.tile([128, 1024], F32, tag="acc")
            both = (slot_a, slot_b)
            for dc in range(NDC):
                for i in range(4):
                    sc = scg * 4 + i
                    nc.tensor.matmul(
                        both[i // 2][:, (i % 2) * 512:(i % 2) * 512 + CL],
                        lhsT=xt_sb[:, dc, sc * 128:(sc + 1) * 128],
                        rhs=wv_sb[:, dc, :],
                        start=(dc == 0), stop=(dc == NDC - 1),
                    )
            for i in range(4):
                sc = scg * 4 + i
                # ACT copy: ScalarE is idle during the projection era and
                # reads PSUM faster than DVE does
                nc.scalar.copy(
                    out=v_sb[:, sc, :, 0:DK],
                    in_=both[i // 2][:, (i % 2) * 512:(i % 2) * 512 + CL]
                    .rearrange("p (h d) -> p h d", h=HEADS_PER_CORE),
                )

        # ---------- attention pieces ----------
        def qk_exp(pair, qh, kc):
            """QK matmuls + exp + band/future fixups; returns the pt tile."""
            w0 = qh * QH
            k0 = kc * 128
            pt = ptp.tile([128, 2, QH], BF16, tag="pt")
            for hh in range(2):
                h = 2 * pair + hh
                p0 = hh * 64
                st = stp.tile([128, QH], F32, tag="st")
                for half in range(2):
                    nc.tensor.matmul(
                        st[:, half * 512:(half + 1) * 512],
                        lhsT=k_sb[p0:p0 + 64, pair, k0:k0 + 128],
                        rhs=q_sb[p0:p0 + 64, pair,
                                 w0 + half * 512:w0 + (half + 1) * 512],
                        start=True, stop=True,
                        tile_position=(p0, 0),
                    )
                nc.scalar.activation(out=pt[:, hh, :], in_=st, func=EXP, scale=SCALE)
                # future region (q <= k0-129): multiply by exp(c_fut - c_past)
                fut_end = min(max(k0 - 128, w0), w0 + QH)
                n_fut = fut_end - w0
                if n_fut > 0:
                    nc.vector.tensor_scalar_mul(
                        out=pt[:, hh, 0:n_fut], in0=pt[:, hh, 0:n_fut],
                        scalar1=fmult_sb[:, h:h + 1],
                    )
                # band: q in [k0-128, k0+256) -> multiply exp(bias - c_past)
                b_lo = max(k0 - 128, w0)
                b_hi = min(k0 + 2 * 128, w0 + QH)
                if b_hi > b_lo:
                    m0 = b_lo - (k0 - 128)
                    nc.vector.tensor_mul(
                        out=pt[:, hh, b_lo - w0:b_hi - w0],
                        in0=pt[:, hh, b_lo - w0:b_hi - w0],
                        in1=band_sb[:, h, m0:m0 + (b_hi - b_lo)],
                    )
            return pt

        def pv(pair, kc, pt, accs):
            for hh in range(2):
                for sub in range(2):
                    nc.tensor.matmul(
                        accs[hh][:, sub * 512:(sub + 1) * 512],
                        lhsT=v_sb[:, kc, 2 * pair + hh, :],
                        rhs=pt[:, hh, sub * 512:(sub + 1) * 512],
                        start=(kc == 0), stop=(kc == N_KC - 1),
                    )

        def evict_and_normalize(gi, pair, qh, accs):
            w0 = qh * QH
            # den copies first so the DRAM round-trip overlaps the ct
            # evictions; the broadcast read is one DMA (nested AP: 2 hh
            # segments x 64 replicated partitions each)
            den_sb = nrm.tile([1, 2, QH], F32, tag="den")
            for hh in range(2):
                nc.vector.tensor_copy(
                    out=den_sb[:, hh, :],
                    in_=accs[hh][DK:DK + 1, :])
            den_v = den_d.ap()
            nc.sync.dma_start(out=den_v[gi], in_=den_sb)
            for hh in range(2):
                nc.vector.tensor_copy(
                    out=ct_sb[hh * 64:hh * 64 + 64, pair, w0:w0 + QH],
                    in_=accs[hh][0:DK, :])
            rbc = nrm.tile([128, QH], F32, tag="rbc")
            bsrc = bass.AP(
                tensor=den_v.tensor,
                offset=den_v.offset + gi * 2 * QH,
                ap=[[QH, 2], [0, 64], [1, QH]],
            )
            nc.sync.dma_start(out=rbc, in_=bsrc)
            nc.vector.reciprocal_approx_fast(out=rbc, in_=rbc)
            nc.vector.tensor_mul(
                out=ct_sb[:, pair, w0:w0 + QH],
                in0=ct_sb[:, pair, w0:w0 + QH],
                in1=rbc,
            )

        def wo_tile(mt, sh):
            # transposed output out^T[m, s]: Wo slices are the stationary
            # operand, ct streams; evictions on ACT (idle in the tail), bf16
            # partials halve the out-DMA bytes.  An (mt, sh) tile needs only
            # the two normalizes of that qh, so sh=0 tiles interleave with
            # the last group's PV block (whose QK slots are free by then).
            ps = stp.tile([128, 1024], F32, tag="st")
            for j in range(2):
                for ss in range(2):
                    nc.tensor.matmul(
                        ps[:, ss * 512:(ss + 1) * 512],
                        lhsT=wo_sb[:, j, mt * 128:(mt + 1) * 128],
                        rhs=ct_sb[:, j, sh * 1024 + ss * 512:
                                  sh * 1024 + (ss + 1) * 512],
                        start=(j == 0), stop=(j == 1),
                    )
            o_sb = outp.tile([128, 1024], BF16, tag="o_sb")
            nc.scalar.copy(out=o_sb, in_=ps)
            nc.sync.dma_start(
                out=out_d.ap()[mt * 128:(mt + 1) * 128,
                               sh * 1024:(sh + 1) * 1024],
                in_=o_sb)

        # ---------- emission schedule ----------
        # group order qh-major: (p0,q0), (p1,q0), (p0,q1), (p1,q1)
        GROUPS = [(0, 0), (1, 0), (0, 1), (1, 1)]

        # K0-t0 and Q0-t0 pace concurrently on the xt stream; group 0's
        # QK+exp starts as soon as both land (pt tiles buffer in the deep
        # ptp pool until PV catches up).
        pair0, qh0 = GROUPS[0]
        g0_pts = []
        qk_flight_t(wk_sb, k_sb, 0, 2, 0)
        qk_flight_t(wq_sb, q_sb, 0, 0, 0)
        for kc in range(8):
            g0_pts.append(qk_exp(pair0, qh0, kc))
        qk_flight_t(wk_sb, k_sb, 0, 2, 1)
        qk_flight_t(wq_sb, q_sb, 0, 0, 1)
        for kc in range(8, N_KC):
            g0_pts.append(qk_exp(pair0, qh0, kc))

        qk_flight(wk_sb, k_sb, 1, 3)
        qk_flight(wq_sb, q_sb, 1, 1)
        for scg in range(4):
            v_flight(scg)

        # Software pipeline, one group deep: group g's PV matmuls interleave
        # with group g+1's QK+exp, so the ACT exp stream never waits behind
        # a block of PV repayment (PE per chunk: 4 PV + 2 QK ~ 1.5us, under
        # the 2.2us exp pace).  The pt pool drains one tile per PV chunk as
        # the next group allocates one, staying at <= 18.
        prev_pts = g0_pts
        prev_pair = pair0
        prev_g = 0
        prev_qh = qh0
        for gi in range(1, 4):
            pair, qh = GROUPS[gi]
            acc_a = accp.tile([DK + 1, QH], F32, tag="acc")
            acc_b = accp.tile([DK + 1, QH], F32, tag="acc")
            accs = [acc_a, acc_b]
            cur_pts = []
            for kc in range(N_KC):
                pv(prev_pair, kc, prev_pts[kc], accs)
                cur_pts.append(qk_exp(pair, qh, kc))
            evict_and_normalize(prev_g, prev_pair, prev_qh, accs)
            prev_pts, prev_pair, prev_g, prev_qh = cur_pts, pair, gi, qh

        # last group's PV, with the qh=0 half of Wo interleaved into it
        acc_a = accp.tile([DK + 1, QH], F32, tag="acc")
        acc_b = accp.tile([DK + 1, QH], F32, tag="acc")
        accs = [acc_a, acc_b]
        for kc in range(N_KC):
            pv(prev_pair, kc, prev_pts[kc], accs)
            if kc % 2 == 1:
                wo_tile(kc // 2, 0)
        evict_and_normalize(prev_g, prev_pair, prev_qh, accs)

        for mt in range(8):
            wo_tile(mt, 1)

    nc.compile()
    return nc


def make_core_inputs(x, Wq, bq, Wk, bk, Wv, bv, Wo, bo, rel_bias):
    """Host-side shard prep. Returns list of 8 in_maps."""
    bf16 = ml_dtypes.bfloat16
    x = np.asarray(x, np.float32)
    in_maps = []
    WqT = np.ascontiguousarray(np.asarray(Wq, np.float32).T.astype(bf16))
    WkT = np.ascontiguousarray(np.asarray(Wk, np.float32).T.astype(bf16))
    WvT = np.ascontiguousarray(np.asarray(Wv, np.float32).T.astype(bf16))
    WoT = np.ascontiguousarray(np.asarray(Wo, np.float32).T.astype(bf16))
    rel = np.asarray(rel_bias, np.float32)
    xt = [np.ascontiguousarray(x[b].T.astype(bf16)) for b in range(B)]

    # band multiplier: [p, h_local, m] = exp(bias(q,k) - c_past), q-k = m-128-p
    p_i = np.arange(128)[:, None]
    m_i = np.arange(BAND)[None, :]
    delta = np.clip(m_i - 128 - p_i, -MAX_REL, MAX_REL) + MAX_REL  # [128, 384]

    for c in range(N_CORES):
        b = c // CORES_PER_BATCH
        g = c % CORES_PER_BATCH
        c0 = g * CL
        heads = np.arange(g * HEADS_PER_CORE, (g + 1) * HEADS_PER_CORE)

        bqk = np.empty((128, 4), np.float32)
        bqk[:, 0] = np.asarray(bq, np.float32)[c0:c0 + 128]
        bqk[:, 1] = np.asarray(bq, np.float32)[c0 + 128:c0 + 256]
        bqk[:, 2] = np.asarray(bk, np.float32)[c0:c0 + 128]
        bqk[:, 3] = np.asarray(bk, np.float32)[c0 + 128:c0 + 256]

        band = np.empty((128, HEADS_PER_CORE, BAND), np.float32)
        fmult = np.empty((128, HEADS_PER_CORE), np.float32)
        for i, hg in enumerate(heads):
            c_past = rel[hg, 2 * MAX_REL]
            band[:, i, :] = np.exp(rel[hg][delta] - c_past)
            fmult[:, i] = np.exp(rel[hg, 0] - c_past)  # future multiplier
        in_maps.append({
            "xt": xt[b],
            "wqt": np.ascontiguousarray(WqT[:, c0:c0 + CL]),
            "wkt": np.ascontiguousarray(WkT[:, c0:c0 + CL]),
            "wvt": np.ascontiguousarray(WvT[:, c0:c0 + CL]),
            "wot": np.ascontiguousarray(WoT[c0:c0 + CL, :]),
            "bqk": bqk,
            "band": band.astype(bf16),
            "fmult": fmult,
        })
    return in_maps


_NC_CACHE = {}


def get_program(**kw):
    key = tuple(sorted(kw.items()))
    if key not in _NC_CACHE:
        _NC_CACHE[key] = build_program(**kw)
    return _NC_CACHE[key]


def kernel(x, Wq, bq, Wk, bk, Wv, bv, Wo, bo, rel_bias):
    from concourse.bass_utils import run_bass_kernel_spmd

    nc = get_program()
    in_maps = make_core_inputs(x, Wq, bq, Wk, bk, Wv, bv, Wo, bo, rel_bias)
    res = run_bass_kernel_spmd(nc, in_maps, core_ids=list(range(N_CORES)))
    results = res.results

    Wo_np = np.asarray(Wo, np.float32)
    const = np.asarray(bv, np.float32) @ Wo_np.T + np.asarray(bo, np.float32)
    out = np.zeros((B, S, D), np.float32)
    for c in range(N_CORES):
        out[c // CORES_PER_BATCH] += results[c]["out_p"].astype(np.float32).T
    out += const[None, None, :]
    return out
